# revision 12
# baseline (speedup 1.0000x reference)
"""MedianFilter1D (k=9, replicate pad) Trainium2 Bass kernel.

Full input x: [8, 32, 131072] f32. Sharded batch-wise across 8 NeuronCores:
core m handles x[m] : [32, 131072], viewed as [128, 32768] (each channel's
L axis split into 4 contiguous segments; rows stay contiguous in L).
The host pre-pads each row with its 4-element halo on both sides
(edge-replicated at channel boundaries) -> x_padded [128, 32776], so every
tile load is a single contiguous-row DMA with no edge special-casing.

Median of 9 = median3( max(lo0,lo1,lo2), med3(mid0,mid1,mid2),
                       min(hi0,hi1,hi2) )
where (lo,mid,hi)_g are the sorted triples of the 3 consecutive groups of 3.
The sliding sort3 is shared across the 3 groups -> 18 min/max ops per output
column instead of ~38 for a full median network.

The toolchain's walrus codegen only accepts ONE sync-wait per instruction,
while Tile's semaphore assignment can emit several; _legalize_multi_waits
rewrites the BIR to hoist all-but-one wait onto single-wait Drain carrier
instructions placed immediately before (same engine => identical blocking
semantics).
"""

import json

import numpy as np

P = 128          # SBUF partitions = rows per core
W = 32768        # columns per row (L / 4 segments)
T = 2048         # tile width (output columns per tile)
H = 4            # halo = (k-1)//2
K = 9
B, C, L = 8, 32, 131072
NCORES = 8
SEG = 4          # segments per channel (W * SEG == L)
WP = W + 2 * H   # padded row length

_cached = {}


# --------------------------------------------------------------------------
# BIR legalization: walrus accepts at most one sync-wait per instruction.
# --------------------------------------------------------------------------

def _legalize_multi_waits(bir_bytes):
    j = json.loads(bir_bytes)
    n_split = 0
    for fn in j.get("functions", []):
        for blk in fn.get("blocks", []):
            insts = blk.get("instructions", [])
            out = []
            for inst in insts:
                si = inst.get("sync_info") or {}
                waits = si.get("on_wait") or []
                if len(waits) > 1:
                    for wi, wv in enumerate(waits[:-1]):
                        n_split += 1
                        out.append({
                            "name": f"{inst['name']}-lw{wi}",
                            "opcode": "Drain",
                            "engine": inst.get("engine", "SP"),
                            "ins": [],
                            "outs": [],
                            "debug": inst.get("debug"),
                            "sync_info": {"on_update": [],
                                          "on_wait": [wv]},
                        })
                    si["on_wait"] = [waits[-1]]
                out.append(inst)
            blk["instructions"] = out
    return json.dumps(j).encode()


def _install_legalizer():
    from concourse import bass2jax, bass_utils
    if getattr(bass_utils, "_mw_legalizer_installed", False):
        return
    orig = bass_utils.compile_bir_kernel

    def wrapped(bir_json, *args, **kwargs):
        return orig(_legalize_multi_waits(bir_json), *args, **kwargs)

    bass_utils.compile_bir_kernel = wrapped
    bass2jax.compile_bir_kernel = wrapped
    bass_utils._mw_legalizer_installed = True


# --------------------------------------------------------------------------
# Kernel build
# --------------------------------------------------------------------------

def _build_nc(reps=1):
    import concourse.bass as bass
    import concourse.mybir as mybir
    from concourse.tile import TileContext

    f32 = mybir.dt.float32
    mn = mybir.AluOpType.min
    mx = mybir.AluOpType.max

    nc = bass.Bass()
    x = nc.dram_tensor("x", [P, WP], f32, kind="ExternalInput")
    y = nc.dram_tensor("y", [P, W], f32, kind="ExternalOutput")

    S = T + 2 * H - 2  # sort3 positions per tile = T + 6
    NT = W // T

    with TileContext(nc) as tc:
        with tc.tile_pool(name="work", bufs=2) as pool:
            v = nc.vector
            for rep in range(reps):
                for it in range(NT):
                    j0 = it * T

                    xt = pool.tile([P, T + 2 * H], f32, tag="xt", bufs=3)
                    nc.sync.dma_start(xt[:, :], x[:, j0:j0 + T + 2 * H])

                    b1 = pool.tile([P, S], f32, tag="b1")
                    b2 = pool.tile([P, S], f32, tag="b2")
                    b3 = pool.tile([P, S], f32, tag="b3")
                    b4 = pool.tile([P, S], f32, tag="b4")
                    b5 = pool.tile([P, S], f32, tag="b5")
                    b6 = pool.tile([P, S], f32, tag="b6")
                    yt = pool.tile([P, T], f32, tag="yt", bufs=3)

                    x0 = xt[:, 0:S]
                    x1 = xt[:, 1:S + 1]
                    x2 = xt[:, 2:S + 2]

                    # stage 1: sliding sort3 -> lo (b3), hi (b5), mid (b6)
                    v.tensor_tensor(b1[:], x0, x1, mn)        # a = min01
                    v.tensor_tensor(b2[:], x0, x1, mx)        # b = max01
                    v.tensor_tensor(b3[:], b1[:], x2, mn)     # lo = min(a,x2)
                    v.tensor_tensor(b4[:], b2[:], x2, mn)     # c = min(b,x2)
                    v.tensor_tensor(b5[:], b2[:], x2, mx)     # hi = max(b,x2)
                    v.tensor_tensor(b6[:], b1[:], b4[:], mx)  # mid = max(a,c)

                    lo0, lo3, lo6 = b3[:, 0:T], b3[:, 3:T + 3], b3[:, 6:T + 6]
                    hi0, hi3, hi6 = b5[:, 0:T], b5[:, 3:T + 3], b5[:, 6:T + 6]
                    md0, md3, md6 = b6[:, 0:T], b6[:, 3:T + 3], b6[:, 6:T + 6]

                    # stage 2
                    m1 = b2[:, 0:T]
                    M1 = b1[:, 0:T]
                    v.tensor_tensor(m1, lo0, lo3, mx)
                    v.tensor_tensor(M1, m1, lo6, mx)          # max of lows

                    m3 = b4[:, 0:T]
                    M3 = b2[:, 0:T]
                    v.tensor_tensor(m3, hi0, hi3, mn)
                    v.tensor_tensor(M3, m3, hi6, mn)          # min of highs

                    d = b3[:, 0:T]   # b3 (lo) dead after M1
                    e = b5[:, 0:T]   # b5 (hi) dead after M3
                    v.tensor_tensor(d, md0, md3, mn)
                    v.tensor_tensor(e, md0, md3, mx)
                    f = b4[:, 0:T]
                    v.tensor_tensor(f, e, md6, mn)
                    M2 = b5[:, 0:T]
                    v.tensor_tensor(M2, d, f, mx)             # med of mids

                    # final med3(M1, M2, M3)
                    g = b4[:, 0:T]
                    h = b6[:, 0:T]
                    v.tensor_tensor(g, M1, M2, mn)
                    v.tensor_tensor(h, M1, M2, mx)
                    i2 = b1[:, 0:T]
                    v.tensor_tensor(i2, h, M3, mn)
                    v.tensor_tensor(yt[:], g, i2, mx)

                    nc.sync.dma_start(y[:, j0:j0 + T], yt[:])
    return nc


def _get_nc(reps=1):
    key = ("nc", reps)
    if key not in _cached:
        _install_legalizer()
        _cached[key] = _build_nc(reps)
    return _cached[key]


def _shard_inputs(x):
    """x: [B, C, L] f32 -> per-core in_maps with rows pre-padded by halo."""
    in_maps = []
    for m in range(NCORES):
        xc = x[m].reshape(P, W)  # row p = c*SEG + s
        xp = np.empty((P, WP), np.float32)
        xp[:, H:H + W] = xc
        s_idx = np.arange(P) % SEG
        inner = s_idx > 0
        first = s_idx == 0
        last = s_idx == SEG - 1
        xp[inner, :H] = xc[np.nonzero(inner)[0] - 1, W - H:]
        xp[first, :H] = np.repeat(xc[first, 0:1], H, axis=1)
        xp[~last, H + W:] = xc[np.nonzero(~last)[0] + 1, :H]
        xp[last, H + W:] = np.repeat(xc[last, W - 1:W], H, axis=1)
        in_maps.append({"x": xp})
    return in_maps


def run_spmd(x, reps=1, **kwargs):
    """Run the bass kernel on 8 cores; returns (out [B,C,L], BassKernelResults)."""
    from concourse import bass_utils
    nc = _get_nc(reps)
    in_maps = _shard_inputs(x)
    res = bass_utils.run_bass_kernel_spmd(
        nc, in_maps, core_ids=list(range(NCORES)), **kwargs)
    out = np.stack([r["y"].reshape(C, L) for r in res.results])
    return out, res


def kernel(x, kernel_size):
    k = int(kernel_size)
    assert k == K, f"kernel built for k={K}, got {k}"
    x = np.asarray(x, dtype=np.float32)
    assert x.shape == (B, C, L)
    out, _ = run_spmd(x)
    return out
